# revision 14
# baseline (speedup 1.0000x reference)
"""Trainium2 Bass kernel for the shared-weight multi-head attention problem.

Math (per batch b, head h, with x_h = x[b,:,h*64:(h+1)*64] [S, d]):
    q = k = x_h @ W + b
    s = q @ q^T / d               (symmetric!)
    t = s + (1-mask_q) * (-1e6)   (constant per softmax row -> only effect is
                                   the fp32 quantization of s for masked rows)
    out_h = softmax(t) @ x_h

Device strategy (8 cores, data parallel over (batch, head-group-of-8)):
  - Keep everything in the "F orientation": tiles [k partitions, q free].
    Because s is symmetric, the same matmul tiles serve softmax row stats
    (via matmul ones-trick) and the PV contraction (k on partitions).
  - No max-subtraction (scores are in [-2, 3]); softmax shift-invariance
    makes this exact to fp rounding.
  - The reference's mask is reproduced exactly by appending two contraction
    rows (+/- 64e6*(1-mask_q)) to the scores matmul: the PE accumulates in
    K order, so fl(s_raw - C) + C applies the same fp32 quantization that
    the reference's `qkt + (-1e6)` does (after the /64 scaling, which is an
    exact power-of-two scale).
  - L_q (softmax denominators) come for free from a ones-column appended to
    the PV stationary operand (M=64 -> 65); a final PE transpose brings the
    output back to natural [q, d] layout with L as column 64, normalized by
    a per-partition reciprocal multiply.
"""

import numpy as np

B, S, D, H, d = 4, 2048, 1024, 16, 64
NH = 8          # heads per core
NCORES = 8
CMASK = np.float32(64.0e6)   # 64 * 1e6 (mask constant scaled to pre-/64 domain)

_NC_CACHE = {}


def _build_nc(s=S, nh=NH):
    import concourse.bacc as bacc
    import concourse.tile as tile
    from concourse import mybir
    from concourse.masks import make_identity

    f32 = mybir.dt.float32
    f32r = mybir.dt.float32r
    Exp = mybir.ActivationFunctionType.Exp

    kt = s // 128          # k tiles per head
    nj = s // 512          # q column blocks per head

    nc = bacc.Bacc("TRN2", target_bir_lowering=False, debug=False)

    x_in = nc.declare_dram_parameter("x", [s, nh * d], f32, isOutput=False)
    w_in = nc.declare_dram_parameter("W", [d, d], f32, isOutput=False)
    b_in = nc.declare_dram_parameter("b", [d, 1], f32, isOutput=False)
    c_in = nc.declare_dram_parameter("crows", [4, s], f32, isOutput=False)
    out = nc.declare_dram_parameter("out", [nh, s, d], f32, isOutput=True)

    with tile.TileContext(nc) as tc:
        with (
            tc.tile_pool(name="const", bufs=1) as const_pool,
            tc.tile_pool(name="xh", bufs=1) as xh_pool,
            tc.tile_pool(name="xhr", bufs=2) as xhr_pool,
            tc.tile_pool(name="xt", bufs=2) as xt_pool,
            tc.tile_pool(name="q", bufs=2) as q_pool,
            tc.tile_pool(name="f", bufs=20) as f_pool,
            tc.tile_pool(name="at", bufs=2) as at_pool,
            tc.tile_pool(name="ob", bufs=4) as ob_pool,
            tc.tile_pool(name="ps", bufs=2, space="PSUM") as ps_pool,
            tc.tile_pool(name="po", bufs=2, space="PSUM") as po_pool,
            tc.tile_pool(name="pm", bufs=2, space="PSUM") as pm_pool,
        ):
            ident = const_pool.tile([128, 128], f32, tag="ident")
            make_identity(nc, ident[:])
            w_raw = const_pool.tile([d, d], f32, tag="wraw")
            nc.sync.dma_start(w_raw[:], w_in[:, :])
            w_sb = const_pool.tile([d, d], f32r, tag="w")
            nc.vector.tensor_copy(w_sb[:], w_raw[:])
            b_sb = const_pool.tile([d, 1], f32, tag="b")
            nc.sync.dma_start(b_sb[:], b_in[:, :])
            # rows: 0 = -C*(1-m), 1 = +C*(1-m), 2,3 = ones
            crows = const_pool.tile([4, s], f32, tag="crows")
            nc.sync.dma_start(crows[:], c_in[:, :])
            ones_sb = const_pool.tile([2, s], f32, tag="ones")
            nc.vector.memset(ones_sb[:], 1.0)

            # x, interleaved with a ones column per 128-row tile: [xh_t | 1]
            xh = []
            for h in range(nh):
                t_ = xh_pool.tile([128, kt * 65], f32, tag=f"xh{h}")
                xh.append(t_)
                for t in range(kt):
                    nc.sync.dma_start(
                        t_[:, t * 65 : t * 65 + 64],
                        x_in[t * 128 : (t + 1) * 128, h * 64 : (h + 1) * 64],
                    )
                nc.vector.memset(t_[:, 64 : kt * 65 : 65], 1.0)

            def emit_head(h):
                # ---- rounded copy of [x_h | 1] for the fp32r PV matmul ----
                xhr = xhr_pool.tile([128, kt * 65], f32r, tag="xhr")
                nc.vector.tensor_copy(xhr[:], xh[h][:])

                # ---- transpose x_h -> xt [d, s] ----
                xt = xt_pool.tile([d, s], f32r, tag="xt")
                for t in range(kt):
                    pt = pm_pool.tile([d, 128], f32, tag="pm")
                    nc.tensor.transpose(
                        pt[:], xh[h][:, t * 65 : t * 65 + 64], ident[:, :]
                    )
                    nc.vector.tensor_copy(xt[:, t * 128 : (t + 1) * 128], pt[:])

                # ---- qT = W^T @ xT + b, two copies with different tail rows:
                # ql rows 64,65 = 1; qr rows 64,65 = -C*(1-m), +C*(1-m)
                ql = q_pool.tile([66, s], f32r, tag="ql")
                qr = q_pool.tile([66, s], f32r, tag="qr")
                nc.vector.tensor_copy(ql[64:66, :], ones_sb[:])
                nc.vector.tensor_copy(qr[64:66, :], crows[0:2, :])
                for j in range(s // 512):
                    pq = pm_pool.tile([d, 512], f32, tag="pm")
                    nc.tensor.matmul(
                        pq[:],
                        w_sb[:],
                        xt[:, j * 512 : (j + 1) * 512],
                        start=True,
                        stop=True,
                    )
                    nc.vector.tensor_scalar_add(
                        ql[0:64, j * 512 : (j + 1) * 512], pq[:], b_sb[:]
                    )
                    nc.vector.tensor_scalar_add(
                        qr[0:64, j * 512 : (j + 1) * 512], pq[:], b_sb[:]
                    )

                # ---- scores -> exp (per q-column block J), then PV ----
                f_of_j = {}

                def emit_scores(J):
                    fs = []
                    for ap_ in range(kt // 2):
                        ps = ps_pool.tile([128, 1024], f32, tag="ps")
                        for u in range(2):
                            a = 2 * ap_ + u
                            nc.tensor.matmul(
                                ps[:, u * 512 : (u + 1) * 512],
                                ql[:, a * 128 : (a + 1) * 128],
                                qr[:, J * 512 : (J + 1) * 512],
                                start=True,
                                stop=True,
                            )
                        ft = f_pool.tile([128, 1024], f32r, tag="F")
                        nc.scalar.activation(
                            ft[:], ps[:], Exp, bias=0.0, scale=1.0 / 64.0
                        )
                        fs.append(ft)
                    f_of_j[J] = fs

                def emit_pv(J):
                    fs = f_of_j.pop(J)
                    po = po_pool.tile([65, 512], f32, tag="po")
                    for t in range(kt):
                        nc.tensor.matmul(
                            po[:],
                            xhr[:, t * 65 : t * 65 + 65],
                            fs[t // 2][:, (t % 2) * 512 : (t % 2 + 1) * 512],
                            start=(t == 0),
                            stop=(t == kt - 1),
                        )
                    at = at_pool.tile([65, 512], f32, tag="at")
                    nc.vector.tensor_copy(at[:], po[:])
                    for u2 in range(4):
                        pn = pm_pool.tile([128, 65], f32, tag="pm")
                        nc.tensor.transpose(
                            pn[:], at[:, u2 * 128 : (u2 + 1) * 128], ident[0:65, 0:65]
                        )
                        rl = ob_pool.tile([128, 1], f32, tag="rl")
                        nc.vector.reciprocal(rl[:], pn[:, 64:65])
                        ob = ob_pool.tile([128, 64], f32, tag="ob")
                        nc.vector.tensor_scalar_mul(ob[:], pn[:, 0:64], rl[:])
                        r0 = J * 512 + u2 * 128
                        nc.sync.dma_start(out[h, r0 : r0 + 128, :], ob[:])

                # software-pipeline: emit scores one J ahead of PV
                for J in range(nj):
                    emit_scores(J)
                    if J >= 1:
                        emit_pv(J - 1)
                emit_pv(nj - 1)

            for h in range(nh):
                emit_head(h)

    nc.compile()
    return nc


def get_nc(s=S, nh=NH):
    key = (s, nh)
    if key not in _NC_CACHE:
        _NC_CACHE[key] = _build_nc(s, nh)
    return _NC_CACHE[key]


def make_in_maps(x, mask, W, b, s=S, nh=NH):
    """Shard full inputs into per-core input maps (core = batch*2 + head_group)."""
    x = np.asarray(x, dtype=np.float32)
    mask = np.asarray(mask)
    W = np.ascontiguousarray(np.asarray(W, dtype=np.float32))
    bv = np.ascontiguousarray(np.asarray(b, dtype=np.float32).reshape(d, 1))
    in_maps = []
    for c in range(NCORES):
        bb, hg = c // 2, c % 2
        xs = np.ascontiguousarray(x[bb, :, hg * nh * d : (hg + 1) * nh * d])
        m1 = np.float32(1.0) - mask[bb].astype(np.float32)
        cneg = (-CMASK * m1).astype(np.float32)
        ones = np.ones_like(cneg)
        crows = np.ascontiguousarray(np.stack([cneg, -cneg, ones, ones], axis=0))
        in_maps.append({"x": xs, "W": W, "b": bv, "crows": crows})
    return in_maps


def gather_out(results):
    """results: list of 8 dicts with 'out' [NH, S, d] -> full [B, S, D]."""
    a = np.empty((B, H, S, d), np.float32)
    for c in range(NCORES):
        bb, hg = c // 2, c % 2
        a[bb, hg * NH : (hg + 1) * NH] = results[c]["out"]
    return a.reshape(B, S, D)


def kernel(x, mask, W, b):
    from concourse.bass_utils import run_bass_kernel_spmd

    nc = get_nc()
    in_maps = make_in_maps(x, mask, W, b)
    res = run_bass_kernel_spmd(nc, in_maps, list(range(NCORES)))
    return gather_out(res.results)


# revision 17
# speedup vs baseline: 15.9104x; 15.9104x over previous
"""Trainium2 Bass kernel for the shared-weight multi-head attention problem.

Math (per batch b, head h, with x_h = x[b,:,h*64:(h+1)*64] [S, d]):
    q = k = x_h @ W + b
    s = q @ q^T / d               (symmetric!)
    t = s + (1-mask_q) * (-1e6)   (constant per softmax row -> only effect is
                                   the fp32 quantization of s for masked rows)
    out_h = softmax(t) @ x_h

Device strategy (8 cores, data parallel over (batch, head-group-of-8)):
  - Keep everything in the "F orientation": tiles [k partitions, q free].
    Because s is symmetric, the same matmul tiles serve softmax row stats
    (via matmul ones-trick) and the PV contraction (k on partitions).
  - No max-subtraction (scores are in [-2, 3]); softmax shift-invariance
    makes this exact to fp rounding.
  - The reference's mask is reproduced exactly by appending two contraction
    rows (+/- 64e6*(1-mask_q)) to the scores matmul: the PE accumulates in
    K order, so fl(s_raw - C) + C applies the same fp32 quantization that
    the reference's `qkt + (-1e6)` does (after the /64 scaling, which is an
    exact power-of-two scale).
  - L_q (softmax denominators) come for free from a ones-column appended to
    the PV stationary operand (M=64 -> 65); a final PE transpose brings the
    output back to natural [q, d] layout with L as column 64, normalized by
    a per-partition reciprocal multiply.
"""

import numpy as np

B, S, D, H, d = 4, 2048, 1024, 16, 64
NH = 8          # heads per core
NCORES = 8
CMASK = np.float32(64.0e6)   # 64 * 1e6 (mask constant scaled to pre-/64 domain)

_NC_CACHE = {}


def _build_nc(s=S, nh=NH):
    import concourse.bacc as bacc
    import concourse.tile as tile
    from concourse import mybir
    from concourse.masks import make_identity

    f32 = mybir.dt.float32
    f32r = mybir.dt.float32r
    bf16 = mybir.dt.bfloat16
    Exp = mybir.ActivationFunctionType.Exp

    kt = s // 128          # k tiles per head
    nj = s // 512          # q column blocks per head

    nc = bacc.Bacc("TRN2", target_bir_lowering=False, debug=False)

    x_in = nc.declare_dram_parameter("x", [s, nh * d], f32, isOutput=False)
    w_in = nc.declare_dram_parameter("W", [d, d], f32, isOutput=False)
    b_in = nc.declare_dram_parameter("b", [d, 1], f32, isOutput=False)
    c_in = nc.declare_dram_parameter("crows", [4, s], f32, isOutput=False)
    out = nc.declare_dram_parameter("out", [nh, s, d], f32, isOutput=True)

    with tile.TileContext(nc) as tc:
        with (
            tc.tile_pool(name="const", bufs=1) as const_pool,
            tc.tile_pool(name="xh", bufs=1) as xh_pool,
            tc.tile_pool(name="xhr", bufs=2) as xhr_pool,
            tc.tile_pool(name="xt", bufs=2) as xt_pool,
            tc.tile_pool(name="q", bufs=2) as q_pool,
            tc.tile_pool(name="f", bufs=20) as f_pool,
            tc.tile_pool(name="at", bufs=2) as at_pool,
            tc.tile_pool(name="ob", bufs=4) as ob_pool,
            tc.tile_pool(name="ps", bufs=2, space="PSUM") as ps_pool,
            tc.tile_pool(name="po", bufs=1, space="PSUM") as po_pool,
            tc.tile_pool(name="pm", bufs=3, space="PSUM") as pm_pool,
        ):
            ident = const_pool.tile([128, 128], f32, tag="ident")
            make_identity(nc, ident[:])
            w_raw = const_pool.tile([d, d], f32, tag="wraw")
            nc.sync.dma_start(w_raw[:], w_in[:, :])
            w_sb = const_pool.tile([d, d], f32r, tag="w")
            nc.vector.tensor_copy(w_sb[:], w_raw[:])
            b_sb = const_pool.tile([d, 1], f32, tag="b")
            nc.sync.dma_start(b_sb[:], b_in[:, :])
            # rows: 0 = -C*(1-m), 1 = +C*(1-m), 2,3 = ones
            crows = const_pool.tile([4, s], f32, tag="crows")
            nc.sync.dma_start(crows[:], c_in[:, :])
            ones_sb = const_pool.tile([2, s], f32, tag="ones")
            nc.vector.memset(ones_sb[:], 1.0)

            # x, interleaved with a ones column per 128-row tile: [xh_t | 1]
            xh = []
            for h in range(nh):
                t_ = xh_pool.tile([128, kt * 65], f32, tag=f"xh{h}")
                xh.append(t_)
                for t in range(kt):
                    nc.sync.dma_start(
                        t_[:, t * 65 : t * 65 + 64],
                        x_in[t * 128 : (t + 1) * 128, h * 64 : (h + 1) * 64],
                    )
                nc.vector.memset(t_[:, 64 : kt * 65 : 65], 1.0)

            def emit_head_prep(h):
                """Transposes + projection for head h; returns tiles for the J loop."""
                # bf16 copy of [x_h | 1] for the PV matmul stationary operand
                xhr = xhr_pool.tile([128, kt * 65], bf16, tag="xhr")
                nc.vector.tensor_copy(xhr[:], xh[h][:])

                # transpose x_h -> xt [d, s]
                xt = xt_pool.tile([d, s], f32r, tag="xt")
                for t in range(kt):
                    pt = pm_pool.tile([d, 128], f32, tag="pm")
                    nc.tensor.transpose(
                        pt[:], xh[h][:, t * 65 : t * 65 + 64], ident[:, :]
                    )
                    nc.vector.tensor_copy(xt[:, t * 128 : (t + 1) * 128], pt[:])

                # qT = W^T @ xT + b, two bf16 copies with different tail rows:
                # ql rows 64,65 = 1; qr rows 64,65 = -C*(1-m), +C*(1-m)
                ql = q_pool.tile([66, s], bf16, tag="ql")
                qr = q_pool.tile([66, s], bf16, tag="qr")
                nc.vector.tensor_copy(ql[64:66, :], ones_sb[:])
                nc.vector.tensor_copy(qr[64:66, :], crows[0:2, :])
                for j in range(s // 512):
                    pq = pm_pool.tile([d, 512], f32, tag="pm")
                    nc.tensor.matmul(
                        pq[:],
                        w_sb[:],
                        xt[:, j * 512 : (j + 1) * 512],
                        start=True,
                        stop=True,
                    )
                    nc.vector.tensor_scalar_add(
                        ql[0:64, j * 512 : (j + 1) * 512], pq[:], b_sb[:]
                    )
                    nc.vector.tensor_scalar_add(
                        qr[0:64, j * 512 : (j + 1) * 512], pq[:], b_sb[:]
                    )
                return xhr, ql, qr

            def emit_head_jloop(h, xhr, ql, qr):
                f_of_j = {}

                def emit_scores(J):
                    fs = []
                    for ap_ in range(kt // 2):
                        ps = ps_pool.tile([128, 1024], f32, tag="ps")
                        for u in range(2):
                            a = 2 * ap_ + u
                            nc.tensor.matmul(
                                ps[:, u * 512 : (u + 1) * 512],
                                ql[:, a * 128 : (a + 1) * 128],
                                qr[:, J * 512 : (J + 1) * 512],
                                start=True,
                                stop=True,
                            )
                        ft = f_pool.tile([128, 1024], bf16, tag="F")
                        nc.scalar.activation(
                            ft[:], ps[:], Exp, bias=0.0, scale=1.0 / 64.0
                        )
                        fs.append(ft)
                    f_of_j[J] = fs

                def emit_pv(J):
                    fs = f_of_j.pop(J)
                    po = po_pool.tile([65, 512], f32, tag="po")
                    for t in range(kt):
                        nc.tensor.matmul(
                            po[:],
                            xhr[:, t * 65 : t * 65 + 65],
                            fs[t // 2][:, (t % 2) * 512 : (t % 2 + 1) * 512],
                            start=(t == 0),
                            stop=(t == kt - 1),
                        )
                    at = at_pool.tile([65, 512], f32, tag="at")
                    nc.vector.tensor_copy(at[:], po[:])
                    for u2 in range(4):
                        pn = pm_pool.tile([128, 65], f32, tag="pm")
                        nc.tensor.transpose(
                            pn[:], at[:, u2 * 128 : (u2 + 1) * 128], ident[0:65, 0:65]
                        )
                        rl = ob_pool.tile([128, 1], f32, tag="rl")
                        nc.vector.reciprocal(rl[:], pn[:, 64:65])
                        ob = ob_pool.tile([128, 64], f32, tag="ob")
                        nc.vector.tensor_scalar_mul(ob[:], pn[:, 0:64], rl[:])
                        r0 = J * 512 + u2 * 128
                        nc.sync.dma_start(out[h, r0 : r0 + 128, :], ob[:])

                # software-pipeline: emit scores one J ahead of PV
                for J in range(nj):
                    emit_scores(J)
                    if J >= 1:
                        emit_pv(J - 1)
                emit_pv(nj - 1)

            # head-level software pipeline: prep(h+1) is emitted before
            # jloop(h) so PE/DVE prep work overlaps the ACT-bound J loop.
            prev = None
            for h in range(nh):
                cur = (h, emit_head_prep(h))
                if prev is not None:
                    ph, (xhr, ql, qr) = prev[0], prev[1]
                    emit_head_jloop(ph, xhr, ql, qr)
                prev = cur
            ph, (xhr, ql, qr) = prev[0], prev[1]
            emit_head_jloop(ph, xhr, ql, qr)

    nc.compile()
    return nc


def get_nc(s=S, nh=NH):
    key = (s, nh)
    if key not in _NC_CACHE:
        _NC_CACHE[key] = _build_nc(s, nh)
    return _NC_CACHE[key]


def make_in_maps(x, mask, W, b, s=S, nh=NH):
    """Shard full inputs into per-core input maps (core = batch*2 + head_group)."""
    x = np.asarray(x, dtype=np.float32)
    mask = np.asarray(mask)
    W = np.ascontiguousarray(np.asarray(W, dtype=np.float32))
    bv = np.ascontiguousarray(np.asarray(b, dtype=np.float32).reshape(d, 1))
    in_maps = []
    for c in range(NCORES):
        bb, hg = c // 2, c % 2
        xs = np.ascontiguousarray(x[bb, :, hg * nh * d : (hg + 1) * nh * d])
        m1 = np.float32(1.0) - mask[bb].astype(np.float32)
        cneg = (-CMASK * m1).astype(np.float32)
        ones = np.ones_like(cneg)
        crows = np.ascontiguousarray(np.stack([cneg, -cneg, ones, ones], axis=0))
        in_maps.append({"x": xs, "W": W, "b": bv, "crows": crows})
    return in_maps


def gather_out(results):
    """results: list of 8 dicts with 'out' [NH, S, d] -> full [B, S, D]."""
    a = np.empty((B, H, S, d), np.float32)
    for c in range(NCORES):
        bb, hg = c // 2, c % 2
        a[bb, hg * NH : (hg + 1) * NH] = results[c]["out"]
    return a.reshape(B, S, D)


def kernel(x, mask, W, b):
    from concourse.bass_utils import run_bass_kernel_spmd

    nc = get_nc()
    in_maps = make_in_maps(x, mask, W, b)
    res = run_bass_kernel_spmd(nc, in_maps, list(range(NCORES)))
    return gather_out(res.results)


# revision 27
# speedup vs baseline: 20.1625x; 1.2673x over previous
"""Trainium2 Bass kernel for the shared-weight multi-head attention problem.

Math (per batch b, head h, with x_h = x[b,:,h*64:(h+1)*64] [S, d]):
    q = k = x_h @ W + b
    s = q @ q^T / d               (symmetric!)
    t = s + (1-mask_q) * (-1e6)   (constant per softmax row -> only effect is
                                   the fp32 quantization of s for masked rows)
    out_h = softmax(t) @ x_h

Device strategy (8 cores, data parallel over (batch, head-group-of-8)):
  - Keep everything in the "F orientation": tiles [k partitions, q free].
    Because s is symmetric, the same matmul tiles serve softmax row stats
    (via matmul ones-trick) and the PV contraction (k on partitions).
  - No max-subtraction (scores are in [-2, 3]); softmax shift-invariance
    makes this exact to fp rounding.
  - The reference's mask is reproduced exactly by appending two contraction
    rows (+/- 64e6*(1-mask_q)) to the scores matmul: the PE accumulates in
    K order, so fl(s_raw - C) + C applies the same fp32 quantization that
    the reference's `qkt + (-1e6)` does (after the /64 scaling, which is an
    exact power-of-two scale).
  - L_q (softmax denominators) come for free from a ones-column appended to
    the PV stationary operand (M=64 -> 65); a final PE transpose brings the
    output back to natural [q, d] layout with L as column 64, normalized by
    a per-partition reciprocal multiply.
"""

import numpy as np

B, S, D, H, d = 4, 2048, 1024, 16, 64
NH = 8          # heads per core
NCORES = 8
CMASK = np.float32(64.0e6)   # 64 * 1e6 (mask constant scaled to pre-/64 domain)


def _chat():
    """CMASK after the bf16 rounding the qr tile applies (63963136.0)."""
    import ml_dtypes

    return float(np.float32(CMASK.astype(ml_dtypes.bfloat16)))


_NC_CACHE = {}


def _build_nc(s=S, nh=NH, mixed_js=(1, 2), masked_js=(3,)):
    import concourse.bacc as bacc
    import concourse.tile as tile
    from concourse import mybir
    from concourse.masks import make_identity

    f32 = mybir.dt.float32
    f32r = mybir.dt.float32r
    bf16 = mybir.dt.bfloat16
    Exp = mybir.ActivationFunctionType.Exp

    kt = s // 128          # k tiles per head
    nj = s // 512          # q column blocks per head
    _CHAT = _chat()

    nc = bacc.Bacc("TRN2", target_bir_lowering=False, debug=False)

    x_in = nc.declare_dram_parameter("x", [s, nh * d], f32, isOutput=False)
    w_in = nc.declare_dram_parameter("W", [d, d], f32, isOutput=False)
    b_in = nc.declare_dram_parameter("b", [d, 1], f32, isOutput=False)
    c_in = nc.declare_dram_parameter("crows", [4, s], f32, isOutput=False)
    out = nc.declare_dram_parameter("out", [nh, s, d], f32, isOutput=True)

    with tile.TileContext(nc) as tc:
        with (
            tc.tile_pool(name="const", bufs=1) as const_pool,
            tc.tile_pool(name="xh", bufs=1) as xh_pool,
            tc.tile_pool(name="xhr", bufs=2) as xhr_pool,
            tc.tile_pool(name="xt", bufs=2) as xt_pool,
            tc.tile_pool(name="q", bufs=2) as q_pool,
            tc.tile_pool(name="f", bufs=20) as f_pool,
            tc.tile_pool(name="at", bufs=2) as at_pool,
            tc.tile_pool(name="ob", bufs=4) as ob_pool,
            tc.tile_pool(name="ps", bufs=2, space="PSUM") as ps_pool,
            tc.tile_pool(name="po", bufs=1, space="PSUM") as po_pool,
            tc.tile_pool(name="pm", bufs=3, space="PSUM") as pm_pool,
        ):
            ident = const_pool.tile([128, 128], f32, tag="ident")
            make_identity(nc, ident[:])
            w_raw = const_pool.tile([d, d], f32, tag="wraw")
            nc.sync.dma_start(w_raw[:], w_in[:, :])
            w_sb = const_pool.tile([d, d], f32r, tag="w")
            nc.vector.tensor_copy(w_sb[:], w_raw[:])
            b_sb = const_pool.tile([d, 1], f32, tag="b")
            nc.sync.dma_start(b_sb[:], b_in[:, :])
            # c_in rows: 0 = -C*(1-m), 1 = +C*(1-m); separate base-0 tiles
            # (engine reads must start at a 32-aligned partition)
            crow_neg = const_pool.tile([1, s], f32, tag="cneg")
            nc.sync.dma_start(crow_neg[:], c_in[0:1, :])
            crow_pos = const_pool.tile([1, s], f32, tag="cpos")
            nc.sync.dma_start(crow_pos[:], c_in[1:2, :])
            ones_sb = const_pool.tile([1, s], f32, tag="ones")
            nc.vector.memset(ones_sb[:], 1.0)
            # bf16 operands for the rank-1 mask-restore matmul (mixed J blocks)
            ones_bf = const_pool.tile([1, 128], bf16, tag="onesbf")
            nc.vector.memset(ones_bf[:], 1.0)
            cpos_bf = const_pool.tile([1, s], bf16, tag="cposbf")
            nc.vector.tensor_copy(cpos_bf[:], crow_pos[:])
            # per-partition +C/64 bias for the fully-masked exp restore
            biasq = const_pool.tile([128, 1], f32, tag="biasq")
            nc.vector.memset(biasq[:], _CHAT / 64.0)

            # x, interleaved with a ones column per 128-row tile: [xh_t | 1]
            xh = []
            for h in range(nh):
                t_ = xh_pool.tile([128, kt * 65], f32, tag=f"xh{h}")
                xh.append(t_)
                for t in range(kt):
                    nc.sync.dma_start(
                        t_[:, t * 65 : t * 65 + 64],
                        x_in[t * 128 : (t + 1) * 128, h * 64 : (h + 1) * 64],
                    )
                nc.vector.memset(t_[:, 64 : kt * 65 : 65], 1.0)

            def emit_head_prep(h):
                """Transposes + projection for head h; returns tiles for the J loop."""
                # bf16 copy of [x_h | 1] for the PV matmul stationary operand
                xhr = xhr_pool.tile([128, kt * 65], bf16, tag="xhr")
                nc.vector.tensor_copy(xhr[:], xh[h][:])

                # transpose x_h -> xt [d, s]
                xt = xt_pool.tile([d, s], f32r, tag="xt")
                for t in range(kt):
                    pt = pm_pool.tile([d, 128], f32, tag="pm")
                    nc.tensor.transpose(
                        pt[:], xh[h][:, t * 65 : t * 65 + 64], ident[:, :]
                    )
                    nc.vector.tensor_copy(xt[:, t * 128 : (t + 1) * 128], pt[:])

                # qT = W^T @ xT + b, two bf16 copies with different tail rows:
                # ql row 64 = 1; qr row 64 = -C*(1-m)
                ql = q_pool.tile([65, s], bf16, tag="ql")
                qr = q_pool.tile([65, s], bf16, tag="qr")
                nc.vector.tensor_copy(ql[64:65, :], ones_sb[0:1, :])
                nc.vector.tensor_copy(qr[64:65, :], crow_neg[0:1, :])
                for j in range(s // 512):
                    pq = pm_pool.tile([d, 512], f32, tag="pm")
                    nc.tensor.matmul(
                        pq[:],
                        w_sb[:],
                        xt[:, j * 512 : (j + 1) * 512],
                        start=True,
                        stop=True,
                    )
                    nc.vector.tensor_scalar_add(
                        ql[0:64, j * 512 : (j + 1) * 512], pq[:], b_sb[:]
                    )
                    nc.vector.tensor_scalar_add(
                        qr[0:64, j * 512 : (j + 1) * 512], pq[:], b_sb[:]
                    )
                return xhr, ql, qr

            def emit_head_jloop(h, xhr, ql, qr):
                f_of_j = {}

                def emit_scores(J):
                    mixed = J in mixed_js
                    # fully-masked blocks: drain left scores quantized at -C;
                    # the ACT fused bias restores +C/64 with one rounding.
                    bias = biasq[:] if J in masked_js else 0.0
                    fs = []
                    for ap_ in range(kt // 2):
                        ps = ps_pool.tile([128, 1024], f32, tag="ps")
                        for u in range(2):
                            a = 2 * ap_ + u
                            nc.tensor.matmul(
                                ps[:, u * 512 : (u + 1) * 512],
                                ql[:, a * 128 : (a + 1) * 128],
                                qr[:, J * 512 : (J + 1) * 512],
                                start=True,
                                stop=not mixed,
                            )
                            if mixed:
                                # rank-1 restore: psum += 1 ⊗ (+C*(1-m)); a
                                # separate matmul so the fp32 rounding of the
                                # drain (the mask quantization) happens first.
                                nc.tensor.matmul(
                                    ps[:, u * 512 : (u + 1) * 512],
                                    ones_bf[:, 0:128],
                                    cpos_bf[:, J * 512 : (J + 1) * 512],
                                    start=False,
                                    stop=True,
                                )
                        ft = f_pool.tile([128, 1024], bf16, tag="F")
                        nc.scalar.activation(
                            ft[:], ps[:], Exp, bias=bias, scale=1.0 / 64.0
                        )
                        fs.append(ft)
                    f_of_j[J] = fs

                def emit_pv(J):
                    fs = f_of_j.pop(J)
                    po = po_pool.tile([65, 512], f32, tag="po")
                    for t in range(kt):
                        nc.tensor.matmul(
                            po[:],
                            xhr[:, t * 65 : t * 65 + 65],
                            fs[t // 2][:, (t % 2) * 512 : (t % 2 + 1) * 512],
                            start=(t == 0),
                            stop=(t == kt - 1),
                        )
                    at = at_pool.tile([65, 512], f32, tag="at")
                    nc.vector.tensor_copy(at[:], po[:])
                    for u2 in range(4):
                        pn = pm_pool.tile([128, 65], f32, tag="pm")
                        nc.tensor.transpose(
                            pn[:], at[:, u2 * 128 : (u2 + 1) * 128], ident[0:65, 0:65]
                        )
                        rl = ob_pool.tile([128, 1], f32, tag="rl")
                        nc.vector.reciprocal(rl[:], pn[:, 64:65])
                        ob = ob_pool.tile([128, 64], f32, tag="ob")
                        nc.vector.tensor_scalar_mul(ob[:], pn[:, 0:64], rl[:])
                        r0 = J * 512 + u2 * 128
                        nc.sync.dma_start(out[h, r0 : r0 + 128, :], ob[:])

                # software-pipeline: emit scores one J ahead of PV
                for J in range(nj):
                    emit_scores(J)
                    if J >= 1:
                        emit_pv(J - 1)
                emit_pv(nj - 1)

            # head-level software pipeline: prep(h+1) is emitted before
            # jloop(h) so PE/DVE prep work overlaps the ACT-bound J loop.
            prev = None
            for h in range(nh):
                cur = (h, emit_head_prep(h))
                if prev is not None:
                    ph, (xhr, ql, qr) = prev[0], prev[1]
                    emit_head_jloop(ph, xhr, ql, qr)
                prev = cur
            ph, (xhr, ql, qr) = prev[0], prev[1]
            emit_head_jloop(ph, xhr, ql, qr)

    nc.compile()
    return nc


def get_nc(s=S, nh=NH, mixed_js=(1, 2), masked_js=(3,)):
    key = (s, nh, mixed_js, masked_js)
    if key not in _NC_CACHE:
        _NC_CACHE[key] = _build_nc(s, nh, mixed_js, masked_js)
    return _NC_CACHE[key]


def plan_mask(mask):
    """Per-batch query permutation (unmasked first) + per-J mask classes.

    Sorting queries makes the mask constant within most 512-query column
    blocks, so the reference's fp32 mask quantization can be reproduced with
    block-constant handling; softmax is per-query, so the permutation only
    reorders output rows.
    """
    mask = np.asarray(mask)
    orders = [np.argsort(-mask[bb], kind="stable") for bb in range(mask.shape[0])]
    n1s = [int(mask[bb].sum()) for bb in range(mask.shape[0])]
    nj = S // 512
    if all(512 <= n1 <= S - 512 for n1 in n1s):
        mixed_js, masked_js = (1, 2), (3,)
    else:
        # fallback: every block uses the exact rank-1 restore
        mixed_js, masked_js = tuple(range(nj)), ()
    return orders, mixed_js, masked_js


def make_in_maps(x, mask, W, b, orders, s=S, nh=NH):
    """Shard full inputs into per-core input maps (core = batch*2 + head_group)."""
    x = np.asarray(x, dtype=np.float32)
    mask = np.asarray(mask)
    W = np.ascontiguousarray(np.asarray(W, dtype=np.float32))
    bv = np.ascontiguousarray(np.asarray(b, dtype=np.float32).reshape(d, 1))
    in_maps = []
    for c in range(NCORES):
        bb, hg = c // 2, c % 2
        order = orders[bb]
        xs = np.ascontiguousarray(x[bb][order, hg * nh * d : (hg + 1) * nh * d])
        m1 = np.float32(1.0) - mask[bb][order].astype(np.float32)
        cneg = (-CMASK * m1).astype(np.float32)
        ones = np.ones_like(cneg)
        crows = np.ascontiguousarray(np.stack([cneg, -cneg, ones, ones], axis=0))
        in_maps.append({"x": xs, "W": W, "b": bv, "crows": crows})
    return in_maps


def gather_out(results, orders):
    """results: list of 8 dicts with 'out' [NH, S, d] -> full [B, S, D]."""
    a = np.empty((B, H, S, d), np.float32)
    for c in range(NCORES):
        bb, hg = c // 2, c % 2
        a[bb, hg * NH : (hg + 1) * NH][:, orders[bb], :] = results[c]["out"]
    return a.reshape(B, S, D)


def kernel(x, mask, W, b):
    from concourse.bass_utils import run_bass_kernel_spmd

    orders, mixed_js, masked_js = plan_mask(mask)
    nc = get_nc(mixed_js=mixed_js, masked_js=masked_js)
    in_maps = make_in_maps(x, mask, W, b, orders)
    res = run_bass_kernel_spmd(nc, in_maps, list(range(NCORES)))
    return gather_out(res.results, orders)


# revision 51
# speedup vs baseline: 4921.9718x; 244.1157x over previous
"""Trainium2 Bass kernel for the shared-weight multi-head attention problem.

Math (per batch b, head h, with x_h = x[b,:,h*64:(h+1)*64] [S, d]):
    q = k = x_h @ W + b
    s = q @ q^T / d               (symmetric!)
    t = s + (1-mask_q) * (-1e6)   (constant per softmax row -> only effect is
                                   the fp32 quantization of s for masked rows)
    out_h = softmax(t) @ x_h

Device strategy (8 cores, data parallel over (batch, head-group-of-8)):
  - Keep everything in the "F orientation": tiles [k partitions, q free].
    Because s is symmetric, the same matmul tiles serve softmax row stats
    (via matmul ones-trick) and the PV contraction (k on partitions).
  - No max-subtraction (scores are in [-2, 3]); softmax shift-invariance
    makes this exact to fp rounding.
  - The reference's mask is reproduced exactly by appending two contraction
    rows (+/- 64e6*(1-mask_q)) to the scores matmul: the PE accumulates in
    K order, so fl(s_raw - C) + C applies the same fp32 quantization that
    the reference's `qkt + (-1e6)` does (after the /64 scaling, which is an
    exact power-of-two scale).
  - L_q (softmax denominators) come for free from a ones-column appended to
    the PV stationary operand (M=64 -> 65); a final PE transpose brings the
    output back to natural [q, d] layout with L as column 64, normalized by
    a per-partition reciprocal multiply.
"""

import numpy as np

B, S, D, H, d = 4, 2048, 1024, 16, 64
NH = 8          # heads per core
NCORES = 8
# Mask shift constant in the pre-/64 score domain. Any C with C/4 an even
# integer and C±|s| inside one fp32 binade of ulp 4 reproduces the reference's
# quantization lattice (0.0625 after /64) including tie behavior; 3*2^24 is
# additionally exact in bf16/fp32r, so every operand path carries it exactly.
CMASK = np.float32(3 * 2**24)  # 50331648


def _chat():
    """CMASK after the bf16 rounding the qr tile applies (exact for 3*2^24)."""
    import ml_dtypes

    return float(np.float32(CMASK.astype(ml_dtypes.bfloat16)))


_NC_CACHE = {}


def _build_nc(s=S, nh=NH, win_lo=0, win_hi=0, reps=1, score_dt="f32r",
              pv_dt="bf16"):
    import concourse.bacc as bacc
    import concourse.tile as tile
    from concourse import mybir
    from concourse.masks import make_identity

    f32 = mybir.dt.float32
    f32r = mybir.dt.float32r
    bf16 = mybir.dt.bfloat16
    Exp = mybir.ActivationFunctionType.Exp

    kt = s // 128          # k tiles per head
    nj = s // 512          # q column blocks per head
    _CHAT = _chat()
    assert _CHAT == float(CMASK)

    nc = bacc.Bacc("TRN2", target_bir_lowering=False, debug=False)

    x_in = nc.declare_dram_parameter("x", [s, nh * d], f32, isOutput=False)
    w_in = nc.declare_dram_parameter("W", [d, d], f32, isOutput=False)
    b_in = nc.declare_dram_parameter("b", [d, 1], f32, isOutput=False)
    c_in = nc.declare_dram_parameter("crows", [4, s], f32, isOutput=False)
    out = nc.declare_dram_parameter("out", [nh, s, d], f32, isOutput=True)

    with tile.TileContext(nc) as tc:
        with (
            tc.tile_pool(name="const", bufs=1) as const_pool,
            tc.tile_pool(name="xh", bufs=1) as xh_pool,
            tc.tile_pool(name="xhr", bufs=2) as xhr_pool,
            tc.tile_pool(name="xt", bufs=2) as xt_pool,
            tc.tile_pool(name="q", bufs=2) as q_pool,
            tc.tile_pool(name="f", bufs=20) as f_pool,
            tc.tile_pool(name="at", bufs=2) as at_pool,
            tc.tile_pool(name="ob", bufs=4) as ob_pool,
            tc.tile_pool(name="ps", bufs=2, space="PSUM") as ps_pool,
            tc.tile_pool(name="po", bufs=1, space="PSUM") as po_pool,
            tc.tile_pool(name="pm", bufs=3, space="PSUM") as pm_pool,
        ):
            ident = const_pool.tile([128, 128], f32, tag="ident")
            make_identity(nc, ident[:])
            ident_bf = const_pool.tile([128, 128], bf16, tag="identbf")
            nc.vector.tensor_copy(ident_bf[:], ident[:])
            w_raw = const_pool.tile([d, d], f32, tag="wraw")
            nc.sync.dma_start(w_raw[:], w_in[:, :])
            w_sb = const_pool.tile([d, d], f32r, tag="w")
            nc.vector.tensor_copy(w_sb[:], w_raw[:])
            b_sb = const_pool.tile([d, 1], f32, tag="b")
            nc.sync.dma_start(b_sb[:], b_in[:, :])
            # c_in rows: 0 = -C*(1-m), 1 = +C*(1-m); separate base-0 tiles
            # (engine reads must start at a 32-aligned partition)
            crow_neg = const_pool.tile([1, s], f32, tag="cneg")
            nc.sync.dma_start(crow_neg[:], c_in[0:1, :])
            crow_pos = const_pool.tile([1, s], f32, tag="cpos")
            nc.sync.dma_start(crow_pos[:], c_in[1:2, :])
            ones_sb = const_pool.tile([1, s], f32, tag="ones")
            nc.vector.memset(ones_sb[:], 1.0)
            # per-partition +C/64 bias for the fully-masked exp restore
            biasq = const_pool.tile([128, 1], f32, tag="biasq")
            nc.vector.memset(biasq[:], _CHAT / 64.0)

            # broadcast of +C*(1-m) over the truly-mixed window [win_lo,win_hi)
            # (rank-1 PE matmul ones ⊗ cpos, materialized once to SBUF)
            wn = win_hi - win_lo
            cposB = None
            if wn > 0:
                ones_bf = const_pool.tile([1, 128], bf16, tag="onesbf")
                nc.vector.memset(ones_bf[:], 1.0)
                cpos_bf = const_pool.tile([1, s], bf16, tag="cposbf")
                nc.vector.tensor_copy(cpos_bf[:], crow_pos[:])
                cposB = const_pool.tile([128, wn], f32, tag="cposB")
                pb = pm_pool.tile([128, wn], f32, tag="pm")
                nc.tensor.matmul(
                    pb[:], ones_bf[:, 0:128], cpos_bf[:, win_lo:win_hi],
                    start=True, stop=True,
                )
                nc.vector.tensor_copy(cposB[:], pb[:])

            xh = []

            dt_s = {"f32r": f32r, "bf16": bf16}[score_dt]
            dt_p = {"f32r": f32r, "bf16": bf16}[pv_dt]

            def emit_head_prep(h):
                """Transposes + projection for head h; returns tiles for the J loop."""
                # low-precision copy of [x_h | 1] for the PV stationary operand
                if pv_dt == "bf16":
                    xhb = xhr_pool.tile([128, kt * 65], bf16, tag="xhb")
                    nc.vector.tensor_copy(xhb[:], xh[h][:])
                    xhr = xhb
                else:
                    xhb = xhr_pool.tile([128, kt * 65], bf16, tag="xhb")
                    nc.vector.tensor_copy(xhb[:], xh[h][:])
                    xhr = xhr_pool.tile([128, kt * 65], dt_p, tag="xhr")
                    nc.vector.tensor_copy(xhr[:], xh[h][:])

                # transpose x_h -> xt [d, s] (bf16 input: 1 cyc/row on PE)
                xt = xt_pool.tile([d, s], f32r, tag="xt")
                for t in range(kt):
                    pt = pm_pool.tile([d, 128], bf16, tag="pm")
                    nc.tensor.transpose(
                        pt[:], xhb[:, t * 65 : t * 65 + 64], ident_bf[:, :]
                    )
                    nc.vector.tensor_copy(xt[:, t * 128 : (t + 1) * 128], pt[:])

                # qT = W^T @ xT + b, two copies with different tail rows:
                # ql row 64 = 1; qr row 64 = -C*(1-m)
                ql = q_pool.tile([65, s], dt_s, tag="ql")
                qr = q_pool.tile([65, s], dt_s, tag="qr")
                nc.vector.tensor_copy(ql[64:65, :], ones_sb[0:1, :])
                nc.vector.tensor_copy(qr[64:65, :], crow_neg[0:1, :])
                for j in range(s // 512):
                    pq = pm_pool.tile([d, 512], f32, tag="pm")
                    nc.tensor.matmul(
                        pq[:],
                        w_sb[:],
                        xt[:, j * 512 : (j + 1) * 512],
                        start=True,
                        stop=True,
                    )
                    nc.vector.tensor_scalar_add(
                        ql[0:64, j * 512 : (j + 1) * 512], pq[:], b_sb[:]
                    )
                    nc.vector.tensor_scalar_add(
                        qr[0:64, j * 512 : (j + 1) * 512], pq[:], b_sb[:]
                    )
                return xhr, ql, qr

            def emit_head_jloop(h, xhr, ql, qr):
                f_of_j = {}

                def emit_scores(J):
                    z0, z2 = J * 512, (J + 1) * 512
                    # DVE restore range: truly-mixed columns of this block
                    rr0, rr1 = max(z0, win_lo), min(z2, win_hi)
                    # columns >= b0 are masked for every batch: the ACT fused
                    # bias (+C/64) undoes the -C quantization shift there.
                    b0 = max(z0, win_hi)
                    fs = []
                    for ap_ in range(kt // 2):
                        ps = ps_pool.tile([128, 1024], f32, tag="ps")
                        for u in range(2):
                            a = 2 * ap_ + u
                            nc.tensor.matmul(
                                ps[:, u * 512 : (u + 1) * 512],
                                ql[:, a * 128 : (a + 1) * 128],
                                qr[:, J * 512 : (J + 1) * 512],
                                start=True,
                                stop=True,
                            )
                        if rr0 < rr1:
                            # per-column restore: psum += broadcast(+C*(1-m));
                            # runs after the matmul drain so the fp32 rounding
                            # (the reference's mask quantization) happens first
                            for u in range(2):
                                c0 = u * 512 + (rr0 - z0)
                                nc.vector.tensor_add(
                                    ps[:, c0 : c0 + (rr1 - rr0)],
                                    ps[:, c0 : c0 + (rr1 - rr0)],
                                    cposB[:, rr0 - win_lo : rr1 - win_lo],
                                )
                        ft = f_pool.tile([128, 1024], dt_p, tag="F")
                        if b0 <= z0:
                            nc.scalar.activation(
                                ft[:], ps[:], Exp, bias=biasq[:], scale=1.0 / 64.0
                            )
                        elif b0 >= z2:
                            nc.scalar.activation(
                                ft[:], ps[:], Exp, bias=0.0, scale=1.0 / 64.0
                            )
                        else:
                            # split: [z0,b0) plain, [b0,z2) with +C/64 bias;
                            # strided APs cover both 512-halves in one call
                            r = b0 - z0
                            ps3 = ps[:].rearrange("p (u c) -> p u c", u=2)
                            ft3 = ft[:].rearrange("p (u c) -> p u c", u=2)
                            nc.scalar.activation(
                                ft3[:, :, 0:r], ps3[:, :, 0:r], Exp,
                                bias=0.0, scale=1.0 / 64.0,
                            )
                            nc.scalar.activation(
                                ft3[:, :, r:512], ps3[:, :, r:512], Exp,
                                bias=biasq[:], scale=1.0 / 64.0,
                            )
                        fs.append(ft)
                    f_of_j[J] = fs

                def emit_pv(J):
                    fs = f_of_j.pop(J)
                    po = po_pool.tile([65, 512], f32, tag="po")
                    for t in range(kt):
                        nc.tensor.matmul(
                            po[:],
                            xhr[:, t * 65 : t * 65 + 65],
                            fs[t // 2][:, (t % 2) * 512 : (t % 2 + 1) * 512],
                            start=(t == 0),
                            stop=(t == kt - 1),
                        )
                    at = at_pool.tile([65, 512], f32, tag="at")
                    nc.vector.tensor_copy(at[:], po[:])
                    for u2 in range(4):
                        pn = pm_pool.tile([128, 65], f32, tag="pm")
                        nc.tensor.transpose(
                            pn[:], at[:, u2 * 128 : (u2 + 1) * 128], ident[0:65, 0:65]
                        )
                        rl = ob_pool.tile([128, 1], f32, tag="rl")
                        nc.vector.reciprocal(rl[:], pn[:, 64:65])
                        ob = ob_pool.tile([128, 64], f32, tag="ob")
                        nc.vector.tensor_scalar_mul(ob[:], pn[:, 0:64], rl[:])
                        r0 = J * 512 + u2 * 128
                        nc.sync.dma_start(out[h, r0 : r0 + 128, :], ob[:])

                # software-pipeline: emit scores one J ahead of PV
                for J in range(nj):
                    emit_scores(J)
                    if J >= 1:
                        emit_pv(J - 1)
                emit_pv(nj - 1)

            # reps > 1 repeats the whole body (for timing-by-slope only)
            for _rep in range(reps):
                # load x, interleaved with a ones column per tile: [xh_t | 1]
                xh.clear()
                for h in range(nh):
                    t_ = xh_pool.tile([128, kt * 65], f32, tag=f"xh{h}")
                    xh.append(t_)
                    for t in range(kt):
                        nc.sync.dma_start(
                            t_[:, t * 65 : t * 65 + 64],
                            x_in[t * 128 : (t + 1) * 128, h * 64 : (h + 1) * 64],
                        )
                    nc.vector.memset(t_[:, 64 : kt * 65 : 65], 1.0)

                # head-level software pipeline: prep(h+1) is emitted before
                # jloop(h) so PE/DVE prep work overlaps the ACT-bound J loop.
                prev = None
                for h in range(nh):
                    cur = (h, emit_head_prep(h))
                    if prev is not None:
                        ph, (xhr, ql, qr) = prev[0], prev[1]
                        emit_head_jloop(ph, xhr, ql, qr)
                    prev = cur
                ph, (xhr, ql, qr) = prev[0], prev[1]
                emit_head_jloop(ph, xhr, ql, qr)

    nc.compile()
    return nc


def get_nc(s=S, nh=NH, win_lo=0, win_hi=0, reps=1, score_dt="f32r", pv_dt="bf16"):
    key = (s, nh, win_lo, win_hi, reps, score_dt, pv_dt)
    if key not in _NC_CACHE:
        _NC_CACHE[key] = _build_nc(s, nh, win_lo, win_hi, reps, score_dt, pv_dt)
    return _NC_CACHE[key]


def plan_mask(mask):
    """Per-batch query permutation (unmasked first) + per-J mask classes.

    Sorting queries makes the mask constant within most 512-query column
    blocks, so the reference's fp32 mask quantization can be reproduced with
    block-constant handling; softmax is per-query, so the permutation only
    reorders output rows.
    """
    mask = np.asarray(mask)
    orders = [np.argsort(-mask[bb], kind="stable") for bb in range(mask.shape[0])]
    n1s = [int(mask[bb].sum()) for bb in range(mask.shape[0])]
    lo, hi = min(n1s), max(n1s)
    win_lo = (lo // 128) * 128
    win_hi = -(-hi // 128) * 128
    return orders, win_lo, win_hi


def make_in_maps(x, mask, W, b, orders, s=S, nh=NH):
    """Shard full inputs into per-core input maps (core = batch*2 + head_group)."""
    x = np.asarray(x, dtype=np.float32)
    mask = np.asarray(mask)
    W = np.ascontiguousarray(np.asarray(W, dtype=np.float32))
    bv = np.ascontiguousarray(np.asarray(b, dtype=np.float32).reshape(d, 1))
    in_maps = []
    for c in range(NCORES):
        bb, hg = c // 2, c % 2
        order = orders[bb]
        xs = np.ascontiguousarray(x[bb][order, hg * nh * d : (hg + 1) * nh * d])
        m1 = np.float32(1.0) - mask[bb][order].astype(np.float32)
        cneg = (-CMASK * m1).astype(np.float32)
        ones = np.ones_like(cneg)
        crows = np.ascontiguousarray(np.stack([cneg, -cneg, ones, ones], axis=0))
        in_maps.append({"x": xs, "W": W, "b": bv, "crows": crows})
    return in_maps


def gather_out(results, orders):
    """results: list of 8 dicts with 'out' [NH, S, d] -> full [B, S, D]."""
    a = np.empty((B, H, S, d), np.float32)
    for c in range(NCORES):
        bb, hg = c // 2, c % 2
        a[bb, hg * NH : (hg + 1) * NH][:, orders[bb], :] = results[c]["out"]
    return a.reshape(B, S, D)


def kernel(x, mask, W, b):
    from concourse.bass_utils import run_bass_kernel_spmd

    orders, win_lo, win_hi = plan_mask(mask)
    nc = get_nc(win_lo=win_lo, win_hi=win_hi)
    in_maps = make_in_maps(x, mask, W, b, orders)
    res = run_bass_kernel_spmd(nc, in_maps, list(range(NCORES)))
    return gather_out(res.results, orders)
